# revision 2
# baseline (speedup 1.0000x reference)
"""Bilateral blur v3: depth-guided separable 5-tap blur + contrast blend,
one 1024x1024 image per core on 8 TRN2 NeuronCores.

Speed strategy vs the fp32 baseline:
- weights via shared squared differences: w_k = exp(-(dp_k-dp)^2 * (c/dp^2)
  + lnc_k). The subtraction (cancellation-critical) stays fp32; everything
  after runs in 2-byte dtypes.
- z-chain (a * r2) in bf16 (range needs the 8-bit exponent), products and
  pair-sums in fp16 (needs the precision, values in [0,6]) -> DVE 2x_1p mode.
- center-path (bacc = b + sb, bx = bacc*winv, db = b - bm) in fp32 so that
  depth-isolated pixels (all tap weights underflow to exactly 0) pass the
  center value through BIT-EXACTLY -- the |b-bm|^0.3 blend amplifies any
  rounding at those pixels catastrophically.
- squares/exp/ln on ACT (1 elem/cycle, has slack), fp32 1x mixed ops
  split DVE/Pool (GPSIMD) to balance engine busy times.
- fp16 taps at odd column offsets would break DVE 2x alignment: keep
  odd-shifted copies (hb_odd/hd_odd/a1o) made by ACT Copy.

Layout identical to the baseline: [P=128 partitions, S=8 row-slots, W]
per image; W chunked by wc; vertical taps via slot shifts with 2-row halo
tiles filled by partition-shifted DMAs.
"""

import math

import numpy as np

import concourse.bacc as bacc
import concourse.tile as tile
from concourse import mybir
from concourse.bass_utils import run_bass_kernel_spmd

_orig_get_act_tables = None


def _single_set_act_tables(arch):
    """Pin the ACT table selector to natural_log_exp_and_others (exp, ln,
    square) to avoid ~2.7us table reloads (same hack as baseline)."""
    tables = _orig_get_act_tables(arch)
    keep = "natural_log_exp_and_others"
    assert keep in tables
    return {k: (v if k == keep else set()) for k, v in tables.items()}


F32 = mybir.dt.float32
F16 = mybir.dt.float16
BF16 = mybir.dt.bfloat16
AF = mybir.ActivationFunctionType
ALU = mybir.AluOpType

B, H, W = 8, 1024, 1024
S = 8
EPS2 = 1e-16


def build_program(h, w, wc, dv, sv, dev_exp, dark_eps, c_enh, repeat=1,
                  pool_ops=("bacc", "dacc", "bmacc", "dmacc", "bm", "dm", "s",
                            "dlt"),
                  debug_outs=False, ablate=()):
    p = h // S
    assert h % S == 0 and w % wc == 0
    nchunk = w // wc
    c = 1.0 / (2.0 * dv)
    ln_c = math.log(c)
    lnc = {k: -(k * k) / (2.0 * sv) for k in (1, 2)}
    pe = dev_exp / 2.0
    lnce = math.log(c_enh)

    global _orig_get_act_tables
    if _orig_get_act_tables is None:
        _orig_get_act_tables = bacc.get_activation_tables
        bacc.get_activation_tables = _single_set_act_tables

    nc = bacc.Bacc("TRN2", target_bir_lowering=False, debug=False)
    d_br = nc.dram_tensor("bright", [h, w], F32, kind="ExternalInput")
    d_dk = nc.dram_tensor("dark", [h, w], F32, kind="ExternalInput")
    d_dp = nc.dram_tensor("depths", [h, w], F32, kind="ExternalInput")
    d_out = nc.dram_tensor("out", [h, w], F32, kind="ExternalOutput")
    if debug_outs:
        d_dbg = {
            nm: nc.dram_tensor(nm, [h, w],
                               F16 if nm == "dbg_w0" else
                               BF16 if nm == "dbg_z0" else F32,
                               kind="ExternalOutput")
            for nm in ("dbg_bx", "dbg_dx", "dbg_sw", "dbg_bm", "dbg_dm",
                       "dbg_w0", "dbg_z0")
        }
        dbg_r = {nm: t.rearrange("(p s) w -> p s w", s=S) for nm, t in d_dbg.items()}

    br_r = d_br.rearrange("(p s) w -> p s w", s=S)
    dk_r = d_dk.rearrange("(p s) w -> p s w", s=S)
    dp_r = d_dp.rearrange("(p s) w -> p s w", s=S)
    out_r = d_out.rearrange("(p s) w -> p s w", s=S)

    dve, gps, act = nc.vector, nc.gpsimd, nc.scalar

    def eng(name):
        return gps if name in pool_ops else dve

    with tile.TileContext(nc) as tc:
        with (
            tc.tile_pool(name="consts", bufs=1) as consts,
            tc.tile_pool(name="loads", bufs=2) as loads,
            tc.tile_pool(name="persist", bufs=2) as persist,
            tc.tile_pool(name="scratch", bufs=1) as scratch,
        ):
            def const_tile(val, tg):
                t = consts.tile([p, 1], F32, tag=tg, name=tg)
                dve.memset(t, float(val))
                return t

            c_zero = const_tile(0.0, "c_zero")
            c_one = const_tile(1.0, "c_one")
            c_lnc1 = const_tile(lnc[1], "c_lnc1")
            c_lnc2 = const_tile(lnc[2], "c_lnc2")
            c_ln_c = const_tile(ln_c, "c_ln_c")
            c_eps2 = const_tile(EPS2, "c_eps2")
            c_lnce = const_tile(lnce, "c_lnce")
            zeros32 = consts.tile([p, 2, wc], F32, tag="zeros32")
            dve.memset(zeros32, 0.0)
            zeros16 = consts.tile([p, 2, 2, wc], F16, tag="zeros16")
            dve.memset(zeros16, 0.0)
            prewarm = consts.tile([p, 1], F32, tag="prewarm")
            act.activation(prewarm, c_zero, AF.Square, bias=c_zero)

            def st(shape, dt, tag):
                return scratch.tile(shape, dt, tag=tag, name=tag)

            def load_top_halo(dst, src_main, zt, fill):
                if "nohalo" in ablate:
                    return
                nc.sync.dma_start(out=dst[1:p], in_=src_main(0, p - 1, S - 2, S))
                if fill:
                    nc.sync.dma_start(out=dst[0:1], in_=zt[0:1])

            def load_bot_halo(dst, src_main, zt, fill):
                if "nohalo" in ablate:
                    return
                nc.sync.dma_start(out=dst[0 : p - 1], in_=src_main(1, p, 0, 2))
                if fill:
                    nc.sync.dma_start(out=dst[p - 1 : p], in_=zt[0:1])

            def stage_a(ci):
                c0 = (ci % nchunk) * wc
                s0, s1 = max(0, c0 - 2), min(w, c0 + wc + 2)
                d0, d1 = s0 - (c0 - 2), s1 - (c0 - 2)

                dp_pad = loads.tile([p, S, wc + 4], F32, tag="dp_pad", name="dp_pad")
                br_pad = loads.tile([p, S, wc + 4], F32, tag="br_pad", name="br_pad")
                dk_pad = loads.tile([p, S, wc + 4], F32, tag="dk_pad", name="dk_pad")
                for t_, src in ((dp_pad, dp_r), (br_pad, br_r), (dk_pad, dk_r)):
                    if d0 != 0:
                        dve.memset(t_[:, :, 0:d0], 0.0)
                    if d1 != wc + 4:
                        dve.memset(t_[:, :, d1 : wc + 4], 0.0)
                    nc.sync.dma_start(out=t_[:, :, d0:d1], in_=src[:, :, s0:s1])

                dp_top = loads.tile([p, 2, wc], F32, tag="dp_top", name="dp_top")
                dp_bot = loads.tile([p, 2, wc], F32, tag="dp_bot", name="dp_bot")
                load_top_halo(
                    dp_top,
                    lambda pa, pb_, sa, sb: dp_r[pa:pb_, sa:sb, c0 : c0 + wc],
                    zeros32, True,
                )
                load_bot_halo(
                    dp_bot,
                    lambda pa, pb_, sa, sb: dp_r[pa:pb_, sa:sb, c0 : c0 + wc],
                    zeros32, True,
                )
                yield

                dp_c = dp_pad[:, :, 2 : wc + 2]
                br_c = br_pad[:, :, 2 : wc + 2]
                dk_c = dk_pad[:, :, 2 : wc + 2]

                # fp16 tap sources (even + odd-shifted copies, on ACT)
                hb_pad = st([p, S, wc + 4], F16, "hb_pad")
                hd_pad = st([p, S, wc + 4], F16, "hd_pad")
                cpe = None if "dvecopy" in ablate else act
                if cpe:
                    cpe.activation(hb_pad, br_pad, AF.Copy)
                    cpe.activation(hd_pad, dk_pad, AF.Copy)
                else:
                    dve.tensor_scalar_mul(hb_pad, br_pad, 1.0)
                    dve.tensor_scalar_mul(hd_pad, dk_pad, 1.0)
                hb_odd = st([p, S, wc + 3], F16, "hb_odd")
                hd_odd = st([p, S, wc + 3], F16, "hd_odd")
                if cpe:
                    cpe.activation(hb_odd, hb_pad[:, :, 1 : wc + 4], AF.Copy)
                    cpe.activation(hd_odd, hd_pad[:, :, 1 : wc + 4], AF.Copy)
                else:
                    dve.tensor_scalar_mul(hb_odd, hb_pad[:, :, 1 : wc + 4], 1.0)
                    dve.tensor_scalar_mul(hd_odd, hd_pad[:, :, 1 : wc + 4], 1.0)
                yield

                # r2 = c / dp^2 = exp(-2 ln dp + ln c), bf16
                ldp = st([p, S, wc], F32, "a_scr32")
                act.activation(ldp, dp_c, AF.Ln, bias=c_zero)
                r2 = persist.tile([p, S, wc], BF16, tag="r2", name="r2")
                act.activation(r2, ldp, AF.Exp, bias=c_ln_c, scale=-2.0)
                yield

                # shared squared differences (fp32 sub -> ACT Square -> bf16)
                dlt1 = st([p, S, wc + 3], F32, "a_scr32")
                eng("dlt").tensor_sub(
                    dlt1, dp_pad[:, :, 1 : wc + 4], dp_pad[:, :, 0 : wc + 3]
                )
                a1 = st([p, S, wc + 3], BF16, "a_a1")
                act.activation(a1, dlt1, AF.Square, bias=c_zero)
                a1o = st([p, S, wc + 2], BF16, "a_a1o")
                if "dvecopy" not in ablate:
                    act.activation(a1o, a1[:, :, 1 : wc + 3], AF.Copy)
                else:
                    dve.tensor_scalar_mul(a1o, a1[:, :, 1 : wc + 3], 1.0)
                dlt2 = st([p, S, wc + 2], F32, "a_scr32")
                eng("dlt").tensor_sub(
                    dlt2, dp_pad[:, :, 2 : wc + 4], dp_pad[:, :, 0 : wc + 2]
                )
                a2 = st([p, S, wc + 2], BF16, "a_a2")
                act.activation(a2, dlt2, AF.Square, bias=c_zero)
                yield

                # z_k = a_k * r2 (bf16 2x); tap order (-1, +1, -2, +2)
                z = st([p, 4, S, wc], BF16, "a_z")
                dve.tensor_mul(z[:, 0], a1o[:, :, 0:wc], r2)
                dve.tensor_mul(z[:, 1], a1[:, :, 2 : wc + 2], r2)
                dve.tensor_mul(z[:, 2], a2[:, :, 0:wc], r2)
                dve.tensor_mul(z[:, 3], a2[:, :, 2 : wc + 2], r2)
                yield

                # w_k = exp(-z + lnc_|k|) (fp16 out; batched by |k|)
                wt = st([p, 4, S, wc], F16, "a_wt")
                act.activation(wt[:, 0:2], z[:, 0:2], AF.Exp,
                               bias=c_lnc1, scale=-1.0)
                act.activation(wt[:, 2:4], z[:, 2:4], AF.Exp,
                               bias=c_lnc2, scale=-1.0)
                yield

                # products (fp16 2x); tap k reads pad col x+2+k
                pb = st([p, 4, S, wc], F16, "a_z")
                pd = st([p, 4, S, wc], F16, "a_pd")
                dve.tensor_mul(pb[:, 0], wt[:, 0], hb_odd[:, :, 0:wc])
                dve.tensor_mul(pb[:, 1], wt[:, 1], hb_odd[:, :, 2 : wc + 2])
                dve.tensor_mul(pb[:, 2], wt[:, 2], hb_pad[:, :, 0:wc])
                dve.tensor_mul(pb[:, 3], wt[:, 3], hb_pad[:, :, 4 : wc + 4])
                yield
                dve.tensor_mul(pd[:, 0], wt[:, 0], hd_odd[:, :, 0:wc])
                dve.tensor_mul(pd[:, 1], wt[:, 1], hd_odd[:, :, 2 : wc + 2])
                dve.tensor_mul(pd[:, 2], wt[:, 2], hd_pad[:, :, 0:wc])
                dve.tensor_mul(pd[:, 3], wt[:, 3], hd_pad[:, :, 4 : wc + 4])
                yield

                # fp16 pair/quad sums; sw promoted to fp32 at the last add
                wsp = st([p, 2, S, wc], F16, "a_wsp")
                dve.tensor_add(wsp, wt[:, 0:2], wt[:, 2:4])
                ub2 = st([p, 2, S, wc], F16, "a_wt")
                dve.tensor_add(ub2, pb[:, 0:2], pb[:, 2:4])
                sb = st([p, S, wc], F16, "a_sb")
                dve.tensor_add(sb, ub2[:, 0], ub2[:, 1])
                ud2 = st([p, 2, S, wc], F16, "a_ud2")
                dve.tensor_add(ud2, pd[:, 0:2], pd[:, 2:4])
                sd = st([p, S, wc], F16, "a_sd")
                dve.tensor_add(sd, ud2[:, 0], ud2[:, 1])
                sw = st([p, S, wc], F32, "a_sw")
                dve.tensor_add(sw, wsp[:, 0], wsp[:, 1])
                yield

                # winv = 1/(1+sw) = exp(-ln(sw + 1))
                lw = st([p, S, wc], F32, "a_scr32")
                act.activation(lw, sw, AF.Ln, bias=c_one)
                winv = st([p, S, wc], F32, "a_winv")
                act.activation(winv, lw, AF.Exp, bias=c_zero, scale=-1.0)

                # center-path fp32 (exact passthrough at isolated pixels)
                bacc_ = st([p, S, wc], F32, "a_bacc")
                eng("bacc").tensor_add(bacc_, br_c, sb)
                dacc_ = st([p, S, wc], F32, "a_dacc")
                eng("dacc").tensor_add(dacc_, dk_c, sd)
                yield
                bxdx = persist.tile([p, 2, S, wc], F32, tag="bxdx", name="bxdx")
                eng("bx").tensor_mul(bxdx[:, 0], bacc_, winv)
                eng("dx").tensor_mul(bxdx[:, 1], dacc_, winv)
                hbxdx = persist.tile([p, 2, S, wc], F16, tag="hbxdx", name="hbxdx")
                if "dvecopy" not in ablate:
                    act.activation(hbxdx, bxdx, AF.Copy)
                else:
                    dve.tensor_scalar_mul(hbxdx, bxdx, 1.0)

                hx_top = persist.tile([p, 2, 2, wc], F16, tag="hx_top", name="hx_top")
                hx_bot = persist.tile([p, 2, 2, wc], F16, tag="hx_bot", name="hx_bot")
                load_top_halo(
                    hx_top, lambda pa, pb_, sa, sb_: hbxdx[pa:pb_, :, sa:sb_, :],
                    zeros16, True,
                )
                load_bot_halo(
                    hx_bot, lambda pa, pb_, sa, sb_: hbxdx[pa:pb_, :, sa:sb_, :],
                    zeros16, True,
                )
                yield
                if debug_outs:
                    cs = slice(c0, c0 + wc)
                    nc.sync.dma_start(out=dbg_r["dbg_bx"][:, :, cs], in_=bxdx[:, 0])
                    nc.sync.dma_start(out=dbg_r["dbg_dx"][:, :, cs], in_=bxdx[:, 1])
                    nc.sync.dma_start(out=dbg_r["dbg_sw"][:, :, cs], in_=sw)
                    nc.sync.dma_start(out=dbg_r["dbg_w0"][:, :, cs], in_=wt[:, 0])
                    nc.sync.dma_start(out=dbg_r["dbg_z0"][:, :, cs], in_=z[:, 0])
                yield dict(
                    c0=c0, dp_c=dp_c, br_c=br_c, dk_c=dk_c, dp_top=dp_top,
                    dp_bot=dp_bot, r2=r2, bxdx=bxdx, hbxdx=hbxdx,
                    hx_top=hx_top, hx_bot=hx_bot,
                )

            def stage_b(stt):
                if "novpass" in ablate:
                    nc.sync.dma_start(
                        out=out_r[:, :, stt["c0"] : stt["c0"] + wc],
                        in_=stt["bxdx"][:, 0],
                    )
                    return
                dp_c, br_c, dk_c = stt["dp_c"], stt["br_c"], stt["dk_c"]
                dp_top, dp_bot = stt["dp_top"], stt["dp_bot"]
                r2 = stt["r2"]
                bx, dx = stt["bxdx"][:, 0], stt["bxdx"][:, 1]
                hbx, hdx = stt["hbxdx"][:, 0], stt["hbxdx"][:, 1]
                hbx_t, hdx_t = stt["hx_top"][:, 0], stt["hx_top"][:, 1]
                hbx_b, hdx_b = stt["hx_bot"][:, 0], stt["hx_bot"][:, 1]

                # vertical diffs: dv1e[s] = dp(row s) - dp(row s-1), s in 0..8
                dv1 = st([p, 9, wc], F32, "b_scr32")
                eng("dlt").tensor_sub(dv1[:, 1:8], dp_c[:, 1:8, :], dp_c[:, 0:7, :])
                eng("dlt").tensor_sub(dv1[:, 0:1], dp_c[:, 0:1, :], dp_top[:, 1:2, :])
                eng("dlt").tensor_sub(dv1[:, 8:9], dp_bot[:, 0:1, :], dp_c[:, 7:8, :])
                a1v = st([p, 9, wc], BF16, "b_a1")
                act.activation(a1v, dv1, AF.Square, bias=c_zero)
                # dv2e[s] = dp(row s) - dp(row s-2), s in 0..9
                dv2 = st([p, 10, wc], F32, "b_scr32b")
                eng("dlt").tensor_sub(dv2[:, 2:8], dp_c[:, 2:8, :], dp_c[:, 0:6, :])
                eng("dlt").tensor_sub(dv2[:, 0:2], dp_c[:, 0:2, :], dp_top[:, 0:2, :])
                eng("dlt").tensor_sub(dv2[:, 8:10], dp_bot[:, 0:2, :], dp_c[:, 6:8, :])
                a2v = st([p, 10, wc], BF16, "b_a2")
                act.activation(a2v, dv2, AF.Square, bias=c_zero)
                yield

                zv = st([p, 4, S, wc], BF16, "b_z")
                dve.tensor_mul(zv[:, 0], a1v[:, 0:8], r2)
                dve.tensor_mul(zv[:, 1], a1v[:, 1:9], r2)
                dve.tensor_mul(zv[:, 2], a2v[:, 0:8], r2)
                dve.tensor_mul(zv[:, 3], a2v[:, 2:10], r2)
                yield
                wv = st([p, 4, S, wc], F16, "b_wt")
                act.activation(wv[:, 0:2], zv[:, 0:2], AF.Exp,
                               bias=c_lnc1, scale=-1.0)
                act.activation(wv[:, 2:4], zv[:, 2:4], AF.Exp,
                               bias=c_lnc2, scale=-1.0)
                yield

                # products with slot-shifted taps + halo pieces (fp16 2x)
                pbv = st([p, 4, S, wc], F16, "b_z")
                pdv = st([p, 4, S, wc], F16, "b_pd")
                for prod, hx, hx_top_, hx_bot_ in (
                    (pbv, hbx, hbx_t, hbx_b),
                    (pdv, hdx, hdx_t, hdx_b),
                ):
                    yield
                    # tap -1
                    dve.tensor_mul(prod[:, 0, 1:S], wv[:, 0, 1:S], hx[:, 0 : S - 1])
                    dve.tensor_mul(prod[:, 0, 0:1], wv[:, 0, 0:1], hx_top_[:, 1:2])
                    # tap +1
                    dve.tensor_mul(prod[:, 1, 0 : S - 1], wv[:, 1, 0 : S - 1], hx[:, 1:S])
                    dve.tensor_mul(prod[:, 1, S - 1 : S], wv[:, 1, S - 1 : S], hx_bot_[:, 0:1])
                    # tap -2
                    dve.tensor_mul(prod[:, 2, 2:S], wv[:, 2, 2:S], hx[:, 0 : S - 2])
                    dve.tensor_mul(prod[:, 2, 0:2], wv[:, 2, 0:2], hx_top_[:, 0:2])
                    # tap +2
                    dve.tensor_mul(prod[:, 3, 0 : S - 2], wv[:, 3, 0 : S - 2], hx[:, 2:S])
                    dve.tensor_mul(prod[:, 3, S - 2 : S], wv[:, 3, S - 2 : S], hx_bot_[:, 0:2])

                wsp = st([p, 2, S, wc], F16, "b_wsp")
                dve.tensor_add(wsp, wv[:, 0:2], wv[:, 2:4])
                ub2 = st([p, 2, S, wc], F16, "b_wt")
                dve.tensor_add(ub2, pbv[:, 0:2], pbv[:, 2:4])
                sbv = st([p, S, wc], F16, "b_sb")
                dve.tensor_add(sbv, ub2[:, 0], ub2[:, 1])
                ud2 = st([p, 2, S, wc], F16, "b_ud2")
                dve.tensor_add(ud2, pdv[:, 0:2], pdv[:, 2:4])
                sdv = st([p, S, wc], F16, "b_sd")
                dve.tensor_add(sdv, ud2[:, 0], ud2[:, 1])
                swv = st([p, S, wc], F32, "b_sw")
                dve.tensor_add(swv, wsp[:, 0], wsp[:, 1])
                yield
                lwv = st([p, S, wc], F32, "b_lw")
                act.activation(lwv, swv, AF.Ln, bias=c_one)
                winvv = st([p, S, wc], F32, "b_winv")
                act.activation(winvv, lwv, AF.Exp, bias=c_zero, scale=-1.0)

                bmacc = st([p, S, wc], F32, "b_bacc")
                eng("bmacc").tensor_add(bmacc, bx, sbv)
                dmacc = st([p, S, wc], F32, "b_dacc")
                eng("dmacc").tensor_add(dmacc, dx, sdv)
                bm = st([p, S, wc], F32, "b_scr32")
                eng("bm").tensor_mul(bm, bmacc, winvv)
                dm = st([p, S, wc], F32, "b_scr32b")
                eng("dm").tensor_mul(dm, dmacc, winvv)
                yield

                if debug_outs:
                    cs = slice(stt["c0"], stt["c0"] + wc)
                    nc.sync.dma_start(out=dbg_r["dbg_bm"][:, :, cs], in_=bm)
                    nc.sync.dma_start(out=dbg_r["dbg_dm"][:, :, cs], in_=dm)
                if "noblend" in ablate:
                    nc.sync.dma_start(
                        out=out_r[:, :, stt["c0"] : stt["c0"] + wc], in_=bm
                    )
                    return
                # blend, split into independent half-width chains so the
                # long serial ACT<->DVE latency overlaps between halves
                dbdd = st([p, 2, S, wc], F32, "b_pd")
                sqs = st([p, 2, S, wc], F32, "b_z")
                lnsq = st([p, 2, S, wc], F32, "b_pd")
                devs = st([p, 2, S, wc], F16, "b_wsp")
                devd = st([p, S, wc], F16, "b_sb")
                ws = st([p, S, wc], F16, "b_sd")
                lws = st([p, S, wc], F16, "b_lw")
                wsi = st([p, S, wc], F16, "b_ud2")
                f = st([p, S, wc], F16, "b_f")
                s_ = st([p, S, wc], F32, "b_sw")
                fs = st([p, S, wc], F32, "b_winv")
                ot = st([p, S, wc], F32, "b_bacc")
                hw_ = wc // 2
                halves = [slice(hi * hw_, (hi + 1) * hw_) for hi in range(2)]
                steps = [
                    lambda cs: eng("db").tensor_sub(dbdd[:, 0, :, cs], br_c[:, :, cs], bm[:, :, cs]),
                    lambda cs: eng("dd").tensor_sub(dbdd[:, 1, :, cs], dk_c[:, :, cs], dm[:, :, cs]),
                    lambda cs: (act.activation(sqs[:, :, :, cs], dbdd[:, :, :, cs], AF.Square, bias=c_zero)
                                if "actsqs" in ablate else
                                eng("sqs").tensor_mul(sqs[:, :, :, cs], dbdd[:, :, :, cs], dbdd[:, :, :, cs])),
                    lambda cs: act.activation(lnsq[:, :, :, cs], sqs[:, :, :, cs], AF.Ln, bias=c_eps2),
                    lambda cs: act.activation(devs[:, 0, :, cs], lnsq[:, 0, :, cs], AF.Exp,
                                              bias=c_lnce, scale=float(pe)),
                    lambda cs: act.activation(devs[:, 1, :, cs], lnsq[:, 1, :, cs], AF.Exp,
                                              bias=c_zero, scale=float(pe)),
                    lambda cs: dve.tensor_single_scalar(devd[:, :, cs], devs[:, 1, :, cs],
                                                        float(dark_eps), ALU.max),
                    lambda cs: dve.tensor_add(ws[:, :, cs], devs[:, 0, :, cs], devd[:, :, cs]),
                    lambda cs: act.activation(lws[:, :, cs], ws[:, :, cs], AF.Ln, bias=c_zero),
                    lambda cs: act.activation(wsi[:, :, cs], lws[:, :, cs], AF.Exp,
                                              bias=c_zero, scale=-1.0),
                    lambda cs: dve.tensor_mul(f[:, :, cs], devs[:, 0, :, cs], wsi[:, :, cs]),
                    lambda cs: eng("s").tensor_sub(s_[:, :, cs], dk_c[:, :, cs], br_c[:, :, cs]),
                    lambda cs: eng("fs").tensor_mul(fs[:, :, cs], f[:, :, cs], s_[:, :, cs]),
                    lambda cs: eng("ot").tensor_add(ot[:, :, cs], br_c[:, :, cs], fs[:, :, cs]),
                ]
                for si, step in enumerate(steps):
                    for cs in halves:
                        step(cs)
                    if si in (2, 5, 7, 9, 11):
                        yield
                nc.sync.dma_start(
                    out=out_r[:, :, stt["c0"] : stt["c0"] + wc], in_=ot
                )

            total = nchunk * repeat
            st_prev = None
            for ci in range(total + 1):
                ga = stage_a(ci) if ci < total else None
                gb = stage_b(st_prev) if st_prev is not None else None
                res = None
                while ga is not None or gb is not None:
                    if ga is not None:
                        try:
                            v = next(ga)
                            if v is not None:
                                res = v
                        except StopIteration:
                            ga = None
                    if gb is not None:
                        try:
                            next(gb)
                        except StopIteration:
                            gb = None
                st_prev = res

    nc.compile()
    return nc


def kernel(
    bright,
    dark,
    depths,
    depth_variance,
    spatial_variance,
    dev_exponent,
    dark_epsilon,
    contrast_enhance,
):
    br = np.ascontiguousarray(np.asarray(bright, np.float32).reshape(B, H, W))
    dk = np.ascontiguousarray(np.asarray(dark, np.float32).reshape(B, H, W))
    dp = np.ascontiguousarray(np.asarray(depths, np.float32).reshape(B, H, W))
    nc = build_program(
        H, W, 128,
        float(depth_variance), float(spatial_variance), float(dev_exponent),
        float(dark_epsilon), float(contrast_enhance),
    )
    in_maps = [
        {"bright": br[i], "dark": dk[i], "depths": dp[i]} for i in range(B)
    ]
    res = run_bass_kernel_spmd(nc, in_maps, list(range(B)))
    out = np.stack([res.results[i]["out"] for i in range(B)])
    return out.reshape(B, H, W, 1).astype(np.float32)


# revision 3
# speedup vs baseline: 1.1095x; 1.1095x over previous
"""Bilateral blur v3: depth-guided separable 5-tap blur + contrast blend,
one 1024x1024 image per core on 8 TRN2 NeuronCores.

Speed strategy vs the fp32 baseline:
- weights via shared squared differences: w_k = exp(-(dp_k-dp)^2 * (c/dp^2)
  + lnc_k). The subtraction (cancellation-critical) stays fp32; everything
  after runs in 2-byte dtypes.
- z-chain (a * r2) in bf16 (range needs the 8-bit exponent), products and
  pair-sums in fp16 (needs the precision, values in [0,6]) -> DVE 2x_1p mode.
- center-path (bacc = b + sb, bx = bacc*winv, db = b - bm) in fp32 so that
  depth-isolated pixels (all tap weights underflow to exactly 0) pass the
  center value through BIT-EXACTLY -- the |b-bm|^0.3 blend amplifies any
  rounding at those pixels catastrophically.
- squares/exp/ln on ACT (1 elem/cycle, has slack), fp32 1x mixed ops
  split DVE/Pool (GPSIMD) to balance engine busy times.
- fp16 taps at odd column offsets would break DVE 2x alignment: keep
  odd-shifted copies (hb_odd/hd_odd/a1o) made by ACT Copy.

Layout identical to the baseline: [P=128 partitions, S=8 row-slots, W]
per image; W chunked by wc; vertical taps via slot shifts with 2-row halo
tiles filled by partition-shifted DMAs.
"""

import math

import numpy as np

import concourse.bacc as bacc
import concourse.tile as tile
from concourse import mybir
from concourse.bass_utils import run_bass_kernel_spmd

_orig_get_act_tables = None


def _single_set_act_tables(arch):
    """Pin the ACT table selector to natural_log_exp_and_others (exp, ln,
    square) to avoid ~2.7us table reloads (same hack as baseline)."""
    tables = _orig_get_act_tables(arch)
    keep = "natural_log_exp_and_others"
    assert keep in tables
    return {k: (v if k == keep else set()) for k, v in tables.items()}


F32 = mybir.dt.float32
F16 = mybir.dt.float16
BF16 = mybir.dt.bfloat16
AF = mybir.ActivationFunctionType
ALU = mybir.AluOpType

B, H, W = 8, 1024, 1024
S = 8
EPS2 = 1e-16


def build_program(h, w, wc, dv, sv, dev_exp, dark_eps, c_enh, repeat=1,
                  pool_ops=("bacc", "dacc", "bmacc", "dmacc", "dlt"),
                  debug_outs=False, ablate=()):
    p = h // S
    assert h % S == 0 and w % wc == 0
    nchunk = w // wc
    c = 1.0 / (2.0 * dv)
    ln_c = math.log(c)
    lnc = {k: -(k * k) / (2.0 * sv) for k in (1, 2)}
    pe = dev_exp / 2.0
    lnce = math.log(c_enh)

    global _orig_get_act_tables
    if _orig_get_act_tables is None:
        _orig_get_act_tables = bacc.get_activation_tables
        bacc.get_activation_tables = _single_set_act_tables

    nc = bacc.Bacc("TRN2", target_bir_lowering=False, debug=False)
    d_br = nc.dram_tensor("bright", [h, w], F32, kind="ExternalInput")
    d_dk = nc.dram_tensor("dark", [h, w], F32, kind="ExternalInput")
    d_dp = nc.dram_tensor("depths", [h, w], F32, kind="ExternalInput")
    d_out = nc.dram_tensor("out", [h, w], F32, kind="ExternalOutput")
    if debug_outs:
        d_dbg = {
            nm: nc.dram_tensor(nm, [h, w],
                               F16 if nm == "dbg_w0" else
                               BF16 if nm == "dbg_z0" else F32,
                               kind="ExternalOutput")
            for nm in ("dbg_bx", "dbg_dx", "dbg_sw", "dbg_bm", "dbg_dm",
                       "dbg_w0", "dbg_z0")
        }
        dbg_r = {nm: t.rearrange("(p s) w -> p s w", s=S) for nm, t in d_dbg.items()}

    br_r = d_br.rearrange("(p s) w -> p s w", s=S)
    dk_r = d_dk.rearrange("(p s) w -> p s w", s=S)
    dp_r = d_dp.rearrange("(p s) w -> p s w", s=S)
    out_r = d_out.rearrange("(p s) w -> p s w", s=S)

    dve, gps, act = nc.vector, nc.gpsimd, nc.scalar

    def eng(name):
        return gps if name in pool_ops else dve

    with tile.TileContext(nc) as tc:
        with (
            tc.tile_pool(name="consts", bufs=1) as consts,
            tc.tile_pool(name="loads", bufs=2) as loads,
            tc.tile_pool(name="persist", bufs=2) as persist,
            tc.tile_pool(name="scratch", bufs=1) as scratch,
        ):
            def const_tile(val, tg):
                t = consts.tile([p, 1], F32, tag=tg, name=tg)
                dve.memset(t, float(val))
                return t

            c_zero = const_tile(0.0, "c_zero")
            c_one = const_tile(1.0, "c_one")
            c_lnc1 = const_tile(lnc[1], "c_lnc1")
            c_lnc2 = const_tile(lnc[2], "c_lnc2")
            c_ln_c = const_tile(ln_c, "c_ln_c")
            c_eps2 = const_tile(EPS2, "c_eps2")
            c_lnce = const_tile(lnce, "c_lnce")
            zeros32 = consts.tile([p, 2, wc], F32, tag="zeros32")
            dve.memset(zeros32, 0.0)
            zeros16 = consts.tile([p, 2, 2, wc], F16, tag="zeros16")
            dve.memset(zeros16, 0.0)
            prewarm = consts.tile([p, 1], F32, tag="prewarm")
            act.activation(prewarm, c_zero, AF.Square, bias=c_zero)

            def st(shape, dt, tag):
                return scratch.tile(shape, dt, tag=tag, name=tag)

            def load_top_halo(dst, src_main, zt, fill):
                if "nohalo" in ablate:
                    return
                nc.sync.dma_start(out=dst[1:p], in_=src_main(0, p - 1, S - 2, S))
                if fill:
                    nc.sync.dma_start(out=dst[0:1], in_=zt[0:1])

            def load_bot_halo(dst, src_main, zt, fill):
                if "nohalo" in ablate:
                    return
                nc.sync.dma_start(out=dst[0 : p - 1], in_=src_main(1, p, 0, 2))
                if fill:
                    nc.sync.dma_start(out=dst[p - 1 : p], in_=zt[0:1])

            def stage_a(ci):
                c0 = (ci % nchunk) * wc
                s0, s1 = max(0, c0 - 2), min(w, c0 + wc + 2)
                d0, d1 = s0 - (c0 - 2), s1 - (c0 - 2)

                dp_pad = loads.tile([p, S, wc + 4], F32, tag="dp_pad", name="dp_pad")
                br_pad = loads.tile([p, S, wc + 4], F32, tag="br_pad", name="br_pad")
                dk_pad = loads.tile([p, S, wc + 4], F32, tag="dk_pad", name="dk_pad")
                for t_, src in ((dp_pad, dp_r), (br_pad, br_r), (dk_pad, dk_r)):
                    if d0 != 0:
                        dve.memset(t_[:, :, 0:d0], 0.0)
                    if d1 != wc + 4:
                        dve.memset(t_[:, :, d1 : wc + 4], 0.0)
                    nc.sync.dma_start(out=t_[:, :, d0:d1], in_=src[:, :, s0:s1])

                dp_top = loads.tile([p, 2, wc], F32, tag="dp_top", name="dp_top")
                dp_bot = loads.tile([p, 2, wc], F32, tag="dp_bot", name="dp_bot")
                load_top_halo(
                    dp_top,
                    lambda pa, pb_, sa, sb: dp_r[pa:pb_, sa:sb, c0 : c0 + wc],
                    zeros32, True,
                )
                load_bot_halo(
                    dp_bot,
                    lambda pa, pb_, sa, sb: dp_r[pa:pb_, sa:sb, c0 : c0 + wc],
                    zeros32, True,
                )
                yield

                dp_c = dp_pad[:, :, 2 : wc + 2]
                br_c = br_pad[:, :, 2 : wc + 2]
                dk_c = dk_pad[:, :, 2 : wc + 2]

                # fp16 tap sources (even + odd-shifted copies, on ACT)
                hb_pad = st([p, S, wc + 4], F16, "hb_pad")
                hd_pad = st([p, S, wc + 4], F16, "hd_pad")
                cpe = None if "dvecopy" in ablate else act
                if cpe:
                    cpe.activation(hb_pad, br_pad, AF.Copy)
                    cpe.activation(hd_pad, dk_pad, AF.Copy)
                else:
                    dve.tensor_scalar_mul(hb_pad, br_pad, 1.0)
                    dve.tensor_scalar_mul(hd_pad, dk_pad, 1.0)
                hb_odd = st([p, S, wc + 3], F16, "hb_odd")
                hd_odd = st([p, S, wc + 3], F16, "hd_odd")
                if cpe:
                    cpe.activation(hb_odd, hb_pad[:, :, 1 : wc + 4], AF.Copy)
                    cpe.activation(hd_odd, hd_pad[:, :, 1 : wc + 4], AF.Copy)
                else:
                    dve.tensor_scalar_mul(hb_odd, hb_pad[:, :, 1 : wc + 4], 1.0)
                    dve.tensor_scalar_mul(hd_odd, hd_pad[:, :, 1 : wc + 4], 1.0)
                yield

                # r2 = c / dp^2 = exp(-2 ln dp + ln c), bf16
                ldp = st([p, S, wc], F32, "a_scr32")
                act.activation(ldp, dp_c, AF.Ln, bias=c_zero)
                r2 = persist.tile([p, S, wc], BF16, tag="r2", name="r2")
                act.activation(r2, ldp, AF.Exp, bias=c_ln_c, scale=-2.0)
                yield

                # shared squared differences (fp32 sub -> ACT Square -> bf16)
                dlt1 = st([p, S, wc + 3], F32, "a_scr32")
                eng("dlt").tensor_sub(
                    dlt1, dp_pad[:, :, 1 : wc + 4], dp_pad[:, :, 0 : wc + 3]
                )
                a1 = st([p, S, wc + 3], BF16, "a_a1")
                act.activation(a1, dlt1, AF.Square, bias=c_zero)
                a1o = st([p, S, wc + 2], BF16, "a_a1o")
                if "dvecopy" not in ablate:
                    act.activation(a1o, a1[:, :, 1 : wc + 3], AF.Copy)
                else:
                    dve.tensor_scalar_mul(a1o, a1[:, :, 1 : wc + 3], 1.0)
                dlt2 = st([p, S, wc + 2], F32, "a_scr32")
                eng("dlt").tensor_sub(
                    dlt2, dp_pad[:, :, 2 : wc + 4], dp_pad[:, :, 0 : wc + 2]
                )
                a2 = st([p, S, wc + 2], BF16, "a_a2")
                act.activation(a2, dlt2, AF.Square, bias=c_zero)
                yield

                # z_k = a_k * r2 (bf16 2x); tap order (-1, +1, -2, +2)
                z = st([p, 4, S, wc], BF16, "a_z")
                dve.tensor_mul(z[:, 0], a1o[:, :, 0:wc], r2)
                dve.tensor_mul(z[:, 1], a1[:, :, 2 : wc + 2], r2)
                dve.tensor_mul(z[:, 2], a2[:, :, 0:wc], r2)
                dve.tensor_mul(z[:, 3], a2[:, :, 2 : wc + 2], r2)
                yield

                # w_k = exp(-z + lnc_|k|) (fp16 out; batched by |k|)
                wt = st([p, 4, S, wc], F16, "a_wt")
                act.activation(wt[:, 0:2], z[:, 0:2], AF.Exp,
                               bias=c_lnc1, scale=-1.0)
                act.activation(wt[:, 2:4], z[:, 2:4], AF.Exp,
                               bias=c_lnc2, scale=-1.0)
                yield

                # products (fp16 2x); tap k reads pad col x+2+k
                pb = st([p, 4, S, wc], F16, "a_z")
                pd = st([p, 4, S, wc], F16, "a_pd")
                dve.tensor_mul(pb[:, 0], wt[:, 0], hb_odd[:, :, 0:wc])
                dve.tensor_mul(pb[:, 1], wt[:, 1], hb_odd[:, :, 2 : wc + 2])
                dve.tensor_mul(pb[:, 2], wt[:, 2], hb_pad[:, :, 0:wc])
                dve.tensor_mul(pb[:, 3], wt[:, 3], hb_pad[:, :, 4 : wc + 4])
                yield
                dve.tensor_mul(pd[:, 0], wt[:, 0], hd_odd[:, :, 0:wc])
                dve.tensor_mul(pd[:, 1], wt[:, 1], hd_odd[:, :, 2 : wc + 2])
                dve.tensor_mul(pd[:, 2], wt[:, 2], hd_pad[:, :, 0:wc])
                dve.tensor_mul(pd[:, 3], wt[:, 3], hd_pad[:, :, 4 : wc + 4])
                yield

                # fp16 pair/quad sums; sw promoted to fp32 at the last add
                wsp = st([p, 2, S, wc], F16, "a_wsp")
                dve.tensor_add(wsp, wt[:, 0:2], wt[:, 2:4])
                ub2 = st([p, 2, S, wc], F16, "a_wt")
                dve.tensor_add(ub2, pb[:, 0:2], pb[:, 2:4])
                sb = st([p, S, wc], F16, "a_sb")
                dve.tensor_add(sb, ub2[:, 0], ub2[:, 1])
                ud2 = st([p, 2, S, wc], F16, "a_ud2")
                dve.tensor_add(ud2, pd[:, 0:2], pd[:, 2:4])
                sd = st([p, S, wc], F16, "a_sd")
                dve.tensor_add(sd, ud2[:, 0], ud2[:, 1])
                sw = st([p, S, wc], F32, "a_sw")
                dve.tensor_add(sw, wsp[:, 0], wsp[:, 1])
                yield

                # winv = 1/(1+sw) = exp(-ln(sw + 1))
                lw = st([p, S, wc], F32, "a_scr32")
                act.activation(lw, sw, AF.Ln, bias=c_one)
                winv = st([p, S, wc], F32, "a_winv")
                act.activation(winv, lw, AF.Exp, bias=c_zero, scale=-1.0)

                # center-path fp32 (exact passthrough at isolated pixels)
                bacc_ = st([p, S, wc], F32, "a_bacc")
                eng("bacc").tensor_add(bacc_, br_c, sb)
                dacc_ = st([p, S, wc], F32, "a_dacc")
                eng("dacc").tensor_add(dacc_, dk_c, sd)
                yield
                bxdx = persist.tile([p, 2, S, wc], F32, tag="bxdx", name="bxdx")
                eng("bx").tensor_mul(bxdx[:, 0], bacc_, winv)
                eng("dx").tensor_mul(bxdx[:, 1], dacc_, winv)
                hbxdx = persist.tile([p, 2, S, wc], F16, tag="hbxdx", name="hbxdx")
                if "dvecopy" not in ablate:
                    act.activation(hbxdx, bxdx, AF.Copy)
                else:
                    dve.tensor_scalar_mul(hbxdx, bxdx, 1.0)

                hx_top = persist.tile([p, 2, 2, wc], F16, tag="hx_top", name="hx_top")
                hx_bot = persist.tile([p, 2, 2, wc], F16, tag="hx_bot", name="hx_bot")
                load_top_halo(
                    hx_top, lambda pa, pb_, sa, sb_: hbxdx[pa:pb_, :, sa:sb_, :],
                    zeros16, True,
                )
                load_bot_halo(
                    hx_bot, lambda pa, pb_, sa, sb_: hbxdx[pa:pb_, :, sa:sb_, :],
                    zeros16, True,
                )
                yield
                if debug_outs:
                    cs = slice(c0, c0 + wc)
                    nc.sync.dma_start(out=dbg_r["dbg_bx"][:, :, cs], in_=bxdx[:, 0])
                    nc.sync.dma_start(out=dbg_r["dbg_dx"][:, :, cs], in_=bxdx[:, 1])
                    nc.sync.dma_start(out=dbg_r["dbg_sw"][:, :, cs], in_=sw)
                    nc.sync.dma_start(out=dbg_r["dbg_w0"][:, :, cs], in_=wt[:, 0])
                    nc.sync.dma_start(out=dbg_r["dbg_z0"][:, :, cs], in_=z[:, 0])
                yield dict(
                    c0=c0, dp_c=dp_c, br_c=br_c, dk_c=dk_c, dp_top=dp_top,
                    dp_bot=dp_bot, r2=r2, bxdx=bxdx, hbxdx=hbxdx,
                    hx_top=hx_top, hx_bot=hx_bot,
                )

            def stage_b(stt):
                if "novpass" in ablate:
                    nc.sync.dma_start(
                        out=out_r[:, :, stt["c0"] : stt["c0"] + wc],
                        in_=stt["bxdx"][:, 0],
                    )
                    return
                dp_c, br_c, dk_c = stt["dp_c"], stt["br_c"], stt["dk_c"]
                dp_top, dp_bot = stt["dp_top"], stt["dp_bot"]
                r2 = stt["r2"]
                bx, dx = stt["bxdx"][:, 0], stt["bxdx"][:, 1]
                hbx, hdx = stt["hbxdx"][:, 0], stt["hbxdx"][:, 1]
                hbx_t, hdx_t = stt["hx_top"][:, 0], stt["hx_top"][:, 1]
                hbx_b, hdx_b = stt["hx_bot"][:, 0], stt["hx_bot"][:, 1]

                # vertical diffs: dv1e[s] = dp(row s) - dp(row s-1), s in 0..8
                dv1 = st([p, 9, wc], F32, "b_scr32")
                eng("dlt").tensor_sub(dv1[:, 1:8], dp_c[:, 1:8, :], dp_c[:, 0:7, :])
                eng("dlt").tensor_sub(dv1[:, 0:1], dp_c[:, 0:1, :], dp_top[:, 1:2, :])
                eng("dlt").tensor_sub(dv1[:, 8:9], dp_bot[:, 0:1, :], dp_c[:, 7:8, :])
                a1v = st([p, 9, wc], BF16, "b_a1")
                act.activation(a1v, dv1, AF.Square, bias=c_zero)
                # dv2e[s] = dp(row s) - dp(row s-2), s in 0..9
                dv2 = st([p, 10, wc], F32, "b_scr32b")
                eng("dlt").tensor_sub(dv2[:, 2:8], dp_c[:, 2:8, :], dp_c[:, 0:6, :])
                eng("dlt").tensor_sub(dv2[:, 0:2], dp_c[:, 0:2, :], dp_top[:, 0:2, :])
                eng("dlt").tensor_sub(dv2[:, 8:10], dp_bot[:, 0:2, :], dp_c[:, 6:8, :])
                a2v = st([p, 10, wc], BF16, "b_a2")
                act.activation(a2v, dv2, AF.Square, bias=c_zero)
                yield

                zv = st([p, 4, S, wc], BF16, "b_z")
                dve.tensor_mul(zv[:, 0], a1v[:, 0:8], r2)
                dve.tensor_mul(zv[:, 1], a1v[:, 1:9], r2)
                dve.tensor_mul(zv[:, 2], a2v[:, 0:8], r2)
                dve.tensor_mul(zv[:, 3], a2v[:, 2:10], r2)
                yield
                wv = st([p, 4, S, wc], F16, "b_wt")
                act.activation(wv[:, 0:2], zv[:, 0:2], AF.Exp,
                               bias=c_lnc1, scale=-1.0)
                act.activation(wv[:, 2:4], zv[:, 2:4], AF.Exp,
                               bias=c_lnc2, scale=-1.0)
                yield

                # products with slot-shifted taps + halo pieces (fp16 2x)
                pbv = st([p, 4, S, wc], F16, "b_z")
                pdv = st([p, 4, S, wc], F16, "b_pd")
                for prod, hx, hx_top_, hx_bot_ in (
                    (pbv, hbx, hbx_t, hbx_b),
                    (pdv, hdx, hdx_t, hdx_b),
                ):
                    yield
                    # tap -1
                    dve.tensor_mul(prod[:, 0, 1:S], wv[:, 0, 1:S], hx[:, 0 : S - 1])
                    dve.tensor_mul(prod[:, 0, 0:1], wv[:, 0, 0:1], hx_top_[:, 1:2])
                    # tap +1
                    dve.tensor_mul(prod[:, 1, 0 : S - 1], wv[:, 1, 0 : S - 1], hx[:, 1:S])
                    dve.tensor_mul(prod[:, 1, S - 1 : S], wv[:, 1, S - 1 : S], hx_bot_[:, 0:1])
                    # tap -2
                    dve.tensor_mul(prod[:, 2, 2:S], wv[:, 2, 2:S], hx[:, 0 : S - 2])
                    dve.tensor_mul(prod[:, 2, 0:2], wv[:, 2, 0:2], hx_top_[:, 0:2])
                    # tap +2
                    dve.tensor_mul(prod[:, 3, 0 : S - 2], wv[:, 3, 0 : S - 2], hx[:, 2:S])
                    dve.tensor_mul(prod[:, 3, S - 2 : S], wv[:, 3, S - 2 : S], hx_bot_[:, 0:2])

                wsp = st([p, 2, S, wc], F16, "b_wsp")
                dve.tensor_add(wsp, wv[:, 0:2], wv[:, 2:4])
                ub2 = st([p, 2, S, wc], F16, "b_wt")
                dve.tensor_add(ub2, pbv[:, 0:2], pbv[:, 2:4])
                sbv = st([p, S, wc], F16, "b_sb")
                dve.tensor_add(sbv, ub2[:, 0], ub2[:, 1])
                ud2 = st([p, 2, S, wc], F16, "b_ud2")
                dve.tensor_add(ud2, pdv[:, 0:2], pdv[:, 2:4])
                sdv = st([p, S, wc], F16, "b_sd")
                dve.tensor_add(sdv, ud2[:, 0], ud2[:, 1])
                swv = st([p, S, wc], F32, "b_sw")
                dve.tensor_add(swv, wsp[:, 0], wsp[:, 1])
                yield
                lwv = st([p, S, wc], F32, "b_lw")
                act.activation(lwv, swv, AF.Ln, bias=c_one)
                winvv = st([p, S, wc], F32, "b_winv")
                act.activation(winvv, lwv, AF.Exp, bias=c_zero, scale=-1.0)

                bmacc = st([p, S, wc], F32, "b_bacc")
                eng("bmacc").tensor_add(bmacc, bx, sbv)
                dmacc = st([p, S, wc], F32, "b_dacc")
                eng("dmacc").tensor_add(dmacc, dx, sdv)
                bm = st([p, S, wc], F32, "b_scr32")
                eng("bm").tensor_mul(bm, bmacc, winvv)
                dm = st([p, S, wc], F32, "b_scr32b")
                eng("dm").tensor_mul(dm, dmacc, winvv)
                yield

                if debug_outs:
                    cs = slice(stt["c0"], stt["c0"] + wc)
                    nc.sync.dma_start(out=dbg_r["dbg_bm"][:, :, cs], in_=bm)
                    nc.sync.dma_start(out=dbg_r["dbg_dm"][:, :, cs], in_=dm)
                if "noblend" in ablate:
                    nc.sync.dma_start(
                        out=out_r[:, :, stt["c0"] : stt["c0"] + wc], in_=bm
                    )
                    return
                # blend, split into independent half-width chains so the
                # long serial ACT<->DVE latency overlaps between halves
                dbdd = st([p, 2, S, wc], F32, "b_pd")
                sqs = st([p, 2, S, wc], F32, "b_z")
                lnsq = st([p, 2, S, wc], F32, "b_pd")
                devs = st([p, 2, S, wc], F16, "b_wsp")
                devd = st([p, S, wc], F16, "b_sb")
                ws = st([p, S, wc], F16, "b_sd")
                lws = st([p, S, wc], F16, "b_lw")
                wsi = st([p, S, wc], F16, "b_ud2")
                f = st([p, S, wc], F16, "b_f")
                s_ = st([p, S, wc], F32, "b_sw")
                fs = st([p, S, wc], F32, "b_winv")
                ot = st([p, S, wc], F32, "b_bacc")
                hw_ = wc // 2
                halves = [slice(hi * hw_, (hi + 1) * hw_) for hi in range(2)]
                steps = [
                    lambda cs: eng("db").tensor_sub(dbdd[:, 0, :, cs], br_c[:, :, cs], bm[:, :, cs]),
                    lambda cs: eng("dd").tensor_sub(dbdd[:, 1, :, cs], dk_c[:, :, cs], dm[:, :, cs]),
                    lambda cs: (act.activation(sqs[:, :, :, cs], dbdd[:, :, :, cs], AF.Square, bias=c_zero)
                                if "dvesqs" not in ablate else
                                eng("sqs").tensor_mul(sqs[:, :, :, cs], dbdd[:, :, :, cs], dbdd[:, :, :, cs])),
                    lambda cs: act.activation(lnsq[:, :, :, cs], sqs[:, :, :, cs], AF.Ln, bias=c_eps2),
                    lambda cs: act.activation(devs[:, 0, :, cs], lnsq[:, 0, :, cs], AF.Exp,
                                              bias=c_lnce, scale=float(pe)),
                    lambda cs: act.activation(devs[:, 1, :, cs], lnsq[:, 1, :, cs], AF.Exp,
                                              bias=c_zero, scale=float(pe)),
                    lambda cs: dve.tensor_single_scalar(devd[:, :, cs], devs[:, 1, :, cs],
                                                        float(dark_eps), ALU.max),
                    lambda cs: dve.tensor_add(ws[:, :, cs], devs[:, 0, :, cs], devd[:, :, cs]),
                    lambda cs: act.activation(lws[:, :, cs], ws[:, :, cs], AF.Ln, bias=c_zero),
                    lambda cs: act.activation(wsi[:, :, cs], lws[:, :, cs], AF.Exp,
                                              bias=c_zero, scale=-1.0),
                    lambda cs: dve.tensor_mul(f[:, :, cs], devs[:, 0, :, cs], wsi[:, :, cs]),
                    lambda cs: eng("s").tensor_sub(s_[:, :, cs], dk_c[:, :, cs], br_c[:, :, cs]),
                    lambda cs: eng("fs").tensor_mul(fs[:, :, cs], f[:, :, cs], s_[:, :, cs]),
                    lambda cs: eng("ot").tensor_add(ot[:, :, cs], br_c[:, :, cs], fs[:, :, cs]),
                ]
                for si, step in enumerate(steps):
                    for cs in halves:
                        step(cs)
                    if si in (2, 5, 7, 9, 11):
                        yield
                nc.sync.dma_start(
                    out=out_r[:, :, stt["c0"] : stt["c0"] + wc], in_=ot
                )

            total = nchunk * repeat
            st_prev = None
            for ci in range(total + 1):
                ga = stage_a(ci) if ci < total else None
                gb = stage_b(st_prev) if st_prev is not None else None
                res = None
                bfirst = "afirst" not in ablate
                while ga is not None or gb is not None:
                    if bfirst and gb is not None:
                        try:
                            next(gb)
                        except StopIteration:
                            gb = None
                    if ga is not None:
                        try:
                            v = next(ga)
                            if v is not None:
                                res = v
                        except StopIteration:
                            ga = None
                    if not bfirst and gb is not None:
                        try:
                            next(gb)
                        except StopIteration:
                            gb = None
                st_prev = res

    nc.compile()
    return nc


def kernel(
    bright,
    dark,
    depths,
    depth_variance,
    spatial_variance,
    dev_exponent,
    dark_epsilon,
    contrast_enhance,
):
    br = np.ascontiguousarray(np.asarray(bright, np.float32).reshape(B, H, W))
    dk = np.ascontiguousarray(np.asarray(dark, np.float32).reshape(B, H, W))
    dp = np.ascontiguousarray(np.asarray(depths, np.float32).reshape(B, H, W))
    nc = build_program(
        H, W, 128,
        float(depth_variance), float(spatial_variance), float(dev_exponent),
        float(dark_epsilon), float(contrast_enhance),
    )
    in_maps = [
        {"bright": br[i], "dark": dk[i], "depths": dp[i]} for i in range(B)
    ]
    res = run_bass_kernel_spmd(nc, in_maps, list(range(B)))
    out = np.stack([res.results[i]["out"] for i in range(B)])
    return out.reshape(B, H, W, 1).astype(np.float32)
